# revision 58
# baseline (speedup 1.0000x reference)
"""Trainium2 Bass kernel for nn_Attention_29738353557815.

8-way tensor-parallel over heads, with the GATE computed post-collective:
  - core c owns q-heads {2c, 2c+1} and kv-head c//2; fp16 datapath
  - pre-collective chain is only kv proj -> q0 proj + h0 attention (-> coll0)
    -> q1 proj + h1 attention (-> coll1); the attention output staged into the
    AllToAll is atr = ot / rowsum (UNGATED) so both collectives launch ~40us
    earlier than a gate-on-owner design
  - the sigmoid gate is computed on the DESTINATION side: each core projects
    gate logits for its OWN 256 tokens x all 16 heads directly in [gcol, t]
    layout (256 small matmuls off hTown; same total PE work as owner-side
    gates since matmul cost is contraction-independent), filling the PE queue
    while the collectives run; sigmoid drains via a one-time ACT table switch
  - o-proj h0-half runs after gate proj (coll0 long done), h1-half right after
    coll1 lands: the PE queue is packed end-to-end and both collectives are
    fully hidden
  - DMA work is spread across the SP/Pool/ACT/DVE queues (each queue is
    charged the transfer time in the cost model): hT+wq+wkv on SP, tables +
    early wo + hTown + first gate-weight blocks on Pool, remaining wo on ACT
    interleaved with the sigmoid drains, remaining gate-weight blocks on
    SP/DVE after staging
  - attention in S^T layout with segment-aligned t-chunks (unchanged from the
    earlier design): causal masks via gpsimd.affine_select for h0, DVE
    threshold-vs-iota for h1 (Pool queue is blocked by coll0 then)
  - norm statistic ops split ACT/DVE (qpre copy on ACT, square on DVE, both
    straight from PSUM) so neither queue backlogs the epilogues
"""
import sys

if "/opt/trn_rl_repo" not in sys.path:
    sys.path.insert(0, "/opt/trn_rl_repo")

import numpy as np

import concourse.bass as bass
from concourse import bacc
import concourse.mybir as mybir
import concourse.tile as tile
from concourse.bass_utils import run_bass_kernel_spmd
from concourse.masks import make_identity

F32 = mybir.dt.float32
F16 = mybir.dt.float16
AF = mybir.ActivationFunctionType
OP = mybir.AluOpType

B, T, D = 1, 2048, 2048
NH, NKV, HD = 16, 4, 128
EPS = 1e-6
SCALE = HD ** -0.5
NCORES = 8
P = 128
NJ = T // 512      # 4 t-chunks of 512
NT = T // P        # 16 s-tiles of 128
DT = D // P        # 16 contraction tiles
TSL = T // NCORES  # 256 output rows per core

_program_cache: dict = {}


def _tile_flags(seg_end: np.ndarray):
    """Segment-aligned attention chunk plan (see earlier design).

    Per chunk: (t0, t1, tiles) with tiles = ((s_tile_i, needs_causal,
    needs_seg, col_off), ...); diagonal tiles width-trimmed, first/last
    full tiles open/close the psum accumulation.
    """
    ends = sorted(set(int(x) for x in seg_end))
    starts = [0] + ends[:-1]
    chunks = []
    for b, e in zip(starts, ends):
        W = e - b
        nch = -(-W // 512)
        base, rem = W // nch, W % nch
        t0 = b
        for k in range(nch):
            t1 = t0 + base + (1 if k < rem else 0)
            full, narrow = [], []
            for i in range(b // P, (t1 - 1) // P + 1):
                smin, smax = P * i, P * i + P - 1
                off = max(0, smin - t0)
                (full if off == 0 else narrow).append(
                    (i, smax >= t0, smin < b, off))
            if len(full) >= 2:
                tiles = [full[0]] + full[1:-1] + narrow + [full[-1]]
            elif narrow:
                ln = narrow[-1]
                tiles = [full[0]] + narrow[:-1] + [(ln[0], ln[1], ln[2], 0)]
            else:
                tiles = full
            chunks.append((t0, t1, tuple(tiles)))
            t0 = t1
    return tuple(chunks)


def _build_program(key, use_collective=True):
    plan, unit_w = key
    NCH = len(plan)
    nc = bacc.Bacc("TRN2", target_bir_lowering=False, debug=False,
                   num_devices=NCORES)

    hT_d = nc.dram_tensor("hT", [P, DT, T], F16, kind="ExternalInput")
    wq_d = nc.dram_tensor("wq", [P, DT, 256], F16, kind="ExternalInput")
    wkv_d = nc.dram_tensor("wkv", [P, DT, 256], F16, kind="ExternalInput")
    wo_d = nc.dram_tensor("wo", [P, NT, 2048], F16, kind="ExternalInput")
    wgp_d = nc.dram_tensor("wgp", [P, NH, DT, P], F16, kind="ExternalInput")
    hto_d = nc.dram_tensor("hto", [P, DT, TSL], F16, kind="ExternalInput")
    tblq_d = nc.dram_tensor("tblq", [2, P, T], F16, kind="ExternalInput")
    if not unit_w:
        wqk_d = nc.dram_tensor("wqk", [P, 2], F16, kind="ExternalInput")
    iota_d = nc.dram_tensor("iota", [P, 512], F16, kind="ExternalInput")
    segrel_d = nc.dram_tensor("segrel", [P, NT, NCH], F16, kind="ExternalInput")
    caurel_d = nc.dram_tensor("caurel", [P, NT, NCH], F16, kind="ExternalInput")
    out_d = nc.dram_tensor("out", [TSL, D], F16, kind="ExternalOutput")

    with tile.TileContext(nc) as tc:
        with (
            tc.tile_pool(name="consts", bufs=1) as consts,
            tc.tile_pool(name="perm", bufs=1) as perm,
            tc.tile_pool(name="hw", bufs=32) as hw,
            tc.tile_pool(name="wop", bufs=3) as wop,
            tc.tile_pool(name="wgs", bufs=8) as wgsp,
            tc.tile_pool(name="tmp", bufs=7) as tmp,
            tc.tile_pool(name="ptp", bufs=7) as ptp,
            tc.tile_pool(name="osb", bufs=8) as osb,
            tc.tile_pool(name="ps", bufs=1, space="PSUM") as psp,
            tc.tile_pool(name="dram", bufs=1, space="DRAM") as dram,
        ):
            # ---- DMA split: first-chunk hT tiles race in on SP+ACT in
            # parallel; hT half-1 split SP(even)/Pool(odd); wq on SP after
            wq_sb = consts.tile([P, DT, 256], F16, tag="wq")
            wkv_sb = [consts.tile([P, 8, 256], F16, tag="wkv", bufs=2,
                                  name=f"wkv{g}") for g in range(2)]

            def wq_ap(dt, col0):
                return wq_sb[:, dt, col0:col0 + 128]

            def wkv_ap(dt, col0):
                return wkv_sb[dt // 8][:, dt % 8, col0:col0 + 128]

            # ---- Pool queue tiles (DMAs emitted in the hT block below in
            # consumption order: tblq, hT1-odd, mask tables, gate/wo)
            tb = {}
            for nm, idx in (("cq", 0), ("sq", 1)):
                t_ = consts.tile([P, T], F16, tag=f"tb_{nm}", name=f"tb_{nm}")
                tb[nm] = t_
            iota_sb = consts.tile([P, 512], F16)
            segrel_sb = consts.tile([P, NT, NCH], F16)
            caurel_sb = consts.tile([P, NT, NCH], F16)
            if not unit_w:
                wqk_sb = consts.tile([P, 2], F16)
            ones_f32 = consts.tile([P, P], F32)
            nc.vector.memset(ones_f32[:], 1.0)
            ones_sb = consts.tile([P, P], F16)
            nc.vector.tensor_copy(ones_sb[:], ones_f32[:])
            ident_f32 = consts.tile([P, P], F32)
            make_identity(nc, ident_f32[:])
            ident_sb = consts.tile([P, P], F16)
            nc.vector.tensor_copy(ident_sb[:], ident_f32[:])
            eps_sb = consts.tile([P, 1], F32)
            nc.vector.memset(eps_sb[:], EPS)
            # prime the ACT table load at t~0 (dedupe keeps only this one
            # until the sigmoid switch)
            actwarm = consts.tile([P, 1], F32)
            nc.scalar.activation(actwarm[:], eps_sb[:], AF.Copy)

            # ---- persistent activations ----
            qTr = [perm.tile([P, T], F16, tag=f"qTr{h}", name=f"qTr{h}")
                   for h in range(2)]
            kTr = perm.tile([P, T], F16, tag="kTr")
            v_sb = perm.tile([P, NT, P], F16, tag="v_sb")
            sgT = perm.tile([P, NH, TSL], F16, tag="sgT")

            a2a_in = [dram.tile([NCORES * P, TSL], F16, name=f"a2a_in{h}")
                      for h in range(2)]
            a2a_in8 = [a.rearrange("(s r) t -> s r t", r=P) for a in a2a_in]
            a2a_out = [dram.tile([NCORES * P, TSL], F16, name=f"a2a_out{h}")
                       for h in range(2)]

            # ================= hT DMA (SP + ACT + Pool) =================
            # half-0 races in on SP(even dt) + ACT(odd dt) so the first kv
            # chain finishes ~8us; half-1 on SP(even) + Pool(odd)
            hTt = [[None] * DT for _ in range(2)]
            for h2 in range(2):
                for dt in range(DT):
                    t_ = hw.tile([P, 1024], F16, tag="hw",
                                 name=f"hT_{h2}_{dt}")
                    hTt[h2][dt] = t_
            # normal priority everywhere: SP and ACT (HWDGE) run DMAs in
            # emission order; Pool's SWDGE stream also drains normal-prio
            # DMAs in emission order from t=0 (hi-pri is counterproductive
            # there)
            nc.sync.dma_start(wkv_sb[0][:, 0:4, :], wkv_d[:, 0:4, :])
            nc.sync.dma_start(wkv_sb[0][:, 4:8, :], wkv_d[:, 4:8, :])
            # half-0 in 512-col quarters raced on three queues so kv chunk
            # 0's chain finishes ~5us
            q3 = {0: nc.sync, 1: nc.scalar, 2: nc.gpsimd}
            for dt in range(DT):
                q3[dt % 3].dma_start(hTt[0][dt][:, 0:512],
                                     hT_d[:, dt, 0:512])
            nc.sync.dma_start(wkv_sb[1][:], wkv_d[:, 8:16, :])
            for dt in range(DT):            # SP/ACT: half-0 second cols
                (nc.sync if dt % 2 == 0 else nc.scalar).dma_start(
                    hTt[0][dt][:, 512:1024], hT_d[:, dt, 512:1024])
            for half in range(2):           # SP: wq
                nc.sync.dma_start(wq_sb[:, 8 * half:8 * half + 8, :],
                                  wq_d[:, 8 * half:8 * half + 8, :])
            for dt in range(DT):            # SP: all of half-1 (consumed
                nc.sync.dma_start(hTt[1][dt][:],  # only after the q0/attn
                                  hT_d[:, dt, 1024:2048])  # half-0 block)
            # Pool, in consumption order: rope tables, mask tables
            for nm, idx in (("cq", 0), ("sq", 1)):
                nc.gpsimd.dma_start(tb[nm][:], tblq_d[idx])
            nc.gpsimd.dma_start(iota_sb[:], iota_d[:])
            nc.gpsimd.dma_start(segrel_sb[:], segrel_d[:])
            nc.gpsimd.dma_start(caurel_sb[:], caurel_d[:])
            if not unit_w:
                nc.gpsimd.dma_start(wqk_sb[:], wqk_d[:])
            hto_sb = consts.tile([P, DT, TSL], F16, tag="hto")
            wgs = [None] * NH
            wo_slices = [None] * NT

            # ================= attention =================
            def emit_attention(h, ch):
                t0, t1, tiles = plan[ch]
                W = t1 - t0
                last = len(tiles) - 1
                ot_ps = psp.tile([P, W], F32, tag="acc", bufs=4,
                                 name=f"ot_{h}_{ch}")
                rs_ps = psp.tile([P, W], F32, tag="acc", bufs=4,
                                 name=f"rs_{h}_{ch}")
                for idx, (i, needs_c, needs_s, off) in enumerate(tiles):
                    w = W - off
                    st_ps = psp.tile([P, w], F32, tag="mm", bufs=3,
                                     name=f"st_{h}_{ch}_{i}")
                    nc.tensor.matmul(st_ps[:], kTr[:, P * i:P * i + P],
                                     qTr[h][:, t0 + off:t1],
                                     start=True, stop=True)
                    pt = ptp.tile([P, w], F16, tag="pt", name=f"pt_{h}_{ch}_{i}")
                    nc.scalar.activation(pt[:], st_ps[:], AF.Exp)
                    if needs_c:
                        if h == 0:
                            # Pool is free until coll0 dispatches
                            nc.gpsimd.affine_select(
                                out=pt[:], in_=pt[:], pattern=[[1, w]],
                                compare_op=OP.is_ge, fill=0.0,
                                base=t0 + off - P * i, channel_multiplier=-1)
                        else:
                            # coll0 holds the Pool queue -- h1 causal on DVE
                            nc.vector.scalar_tensor_tensor(
                                out=pt[:], in0=iota_sb[:, off:W],
                                scalar=caurel_sb[:, i, ch:ch + 1], in1=pt[:],
                                op0=OP.is_ge, op1=OP.mult)
                    if needs_s:
                        seng = nc.vector
                        seng.scalar_tensor_tensor(
                            out=pt[:], in0=iota_sb[:, off:W],
                            scalar=segrel_sb[:, i, ch:ch + 1], in1=pt[:],
                            op0=OP.is_lt, op1=OP.mult)
                    nc.tensor.matmul(ot_ps[:, off:W], v_sb[:, i, :], pt[:],
                                     start=(idx == 0), stop=(idx == last))
                    nc.tensor.matmul(rs_ps[:, off:W], ones_sb[:], pt[:],
                                     start=(idx == 0), stop=(idx == last))

                # atr = ot / rowsum = ot * exp(-ln(rs)); gate applied
                # post-collective
                den = tmp.tile([P, W], F32, tag="tmp", name=f"den_{h}_{ch}")
                nc.scalar.activation(den[:], rs_ps[:], AF.Ln)
                nc.scalar.activation(den[:], den[:], AF.Exp, scale=-1.0)
                atr = tmp.tile([P, W], F16, tag="tmp2", bufs=2,
                               name=f"atr_{h}_{ch}")
                nc.vector.tensor_tensor(atr[:], ot_ps[:], den[:], OP.mult)
                with tc.high_priority():
                    for r in range(t0 // TSL, (t1 - 1) // TSL + 1):
                        a = max(TSL * r, t0)
                        b = min(TSL * r + TSL, t1)
                        nc.sync.dma_start(
                            a2a_in8[h][r, :, a - TSL * r:b - TSL * r],
                            atr[:, a - t0:b - t0])

            # ================= projections =================
            def emit_proj_mm(j, c):
                # c: 0 = q-head0, 1 = q-head1, 2 = k, 3 = v
                h2, jj = j // 2, j % 2
                hsl = slice(512 * jj, 512 * jj + 512)
                hTj = hTt[h2]
                if c == 0:
                    w_ap = lambda dt: wq_ap(dt, 0)
                elif c == 1:
                    w_ap = lambda dt: wq_ap(dt, 128)
                elif c == 2:
                    w_ap = lambda dt: wkv_ap(dt, 0)
                else:
                    w_ap = lambda dt: wkv_ap(dt, 128)
                ptag, pbufs = ("mm", 3) if c != 3 else ("acc", 4)
                mm_ps = psp.tile([P, 512], F32, tag=ptag, bufs=pbufs,
                                 name=f"proj_{j}_{c}")
                for dt in range(DT):
                    nc.tensor.matmul(mm_ps[:], w_ap(dt), hTj[dt][:, hsl],
                                     start=(dt == 0), stop=(dt == DT - 1))
                return mm_ps

            def emit_norm(j, c, mm_ps, veng=None):
                # q0/q1/k: rms-norm + rope; square on the vector engine off
                # the fp16 copy (ACT only does the rsv Ln/Exp, hi-pri so it
                # schedules ahead of the attention exp backlog)
                veng = veng or nc.vector
                tsl = slice(512 * j, 512 * j + 512)
                dest = qTr[c][:, tsl] if c < 2 else kTr[:, tsl]
                qpre = tmp.tile([P, 512], F16, tag="tmp")
                nc.vector.tensor_copy(qpre[:], mm_ps[:])
                q2 = ptp.tile([P, 512], F16, tag="pt")
                nc.vector.tensor_tensor(q2[:], mm_ps[:], qpre[:], OP.mult)
                if not unit_w:
                    qw = tmp.tile([P, 512], F16, tag="tmp")
                    veng.tensor_scalar_mul(
                        qw[:], qpre[:],
                        wqk_sb[:, (0 if c < 2 else 1):(1 if c < 2 else 2)])
                    qpre = qw
                ssq_ps = psp.tile([P, 512], F32, tag="aux", bufs=1)
                nc.tensor.matmul(ssq_ps[:], ones_sb[:], q2[:],
                                 start=True, stop=True)
                rsv = tmp.tile([P, 512], F16, tag="tmp")
                with tc.high_priority():
                    nc.scalar.activation(rsv[:], ssq_ps[:], AF.Ln,
                                         scale=1.0 / HD, bias=eps_sb[:, 0:1])
                    nc.scalar.activation(rsv[:], rsv[:], AF.Exp, scale=-0.5)
                tcos = tmp.tile([P, 512], F16, tag="tmp")
                veng.tensor_tensor(tcos[:], qpre[:], tb["cq"][:, tsl],
                                   OP.mult)
                t2 = tmp.tile([P, 512], F16, tag="tmp")
                # sin halves pre-swapped host-side
                veng.tensor_tensor(t2[0:64, :], qpre[64:128, :],
                                   tb["sq"][64:128, tsl], OP.mult)
                veng.tensor_tensor(t2[64:128, :], qpre[0:64, :],
                                   tb["sq"][0:64, tsl], OP.mult)
                veng.tensor_tensor(t2[:], tcos[:], t2[:], OP.add)
                veng.tensor_tensor(dest, t2[:], rsv[:], OP.mult)

            def emit_vtrans(j, mm_ps, veng=None):
                veng = veng or nc.vector
                vtmp = tmp.tile([P, 512], F16, tag="tmp")
                nc.scalar.activation(vtmp[:], mm_ps[:], AF.Copy)
                for kk in range(4):
                    tt = 4 * j + kk
                    trp = psp.tile([P, P], F16, tag="aux", bufs=1)
                    nc.tensor.transpose(
                        trp[:], vtmp[:, 128 * kk:128 * kk + 128],
                        ident_sb[:])
                    nc.vector.tensor_copy(v_sb[:, tt, :], trp[:])

            def emit_gate(g, exp_chain=False):
                # gate logits for own tokens, head-column g, in [gcol, t]
                # layout off hTown; filler tiles inside the attention phases
                # use the Exp/Ln sigmoid so the ACT table never switches
                gt_ps = psp.tile([P, TSL], F32, tag="mm", bufs=3,
                                 name=f"gt_{g}")
                for dt in range(DT):
                    nc.tensor.matmul(
                        gt_ps[:], wgs[g][:, dt, :], hto_sb[:, dt, :],
                        start=(dt == 0), stop=(dt == DT - 1))
                if not exp_chain:
                    nc.scalar.activation(sgT[:, g, :], gt_ps[:], AF.Sigmoid)
                else:
                    # sig(x) = exp(-ln(1 + exp(-x)))
                    eg = tmp.tile([P, TSL], F16, tag="tmp", name=f"eg{g}")
                    nc.scalar.activation(eg[:], gt_ps[:], AF.Exp, scale=-1.0)
                    e1 = tmp.tile([P, TSL], F16, tag="tmp", name=f"e1{g}")
                    nc.vector.tensor_scalar_add(e1[:], eg[:], 1.0)
                    nc.scalar.activation(e1[:], e1[:], AF.Ln)
                    nc.scalar.activation(sgT[:, g, :], e1[:], AF.Exp,
                                         scale=-1.0)

            # attention chunk ch ready once proj chunk (t1-1)//512 is done
            ch_by_j = [[] for _ in range(NJ)]
            for ch, (t0, t1, _) in enumerate(plan):
                ch_by_j[(t1 - 1) // 512].append(ch)
            for grp in ch_by_j:
                grp.sort(key=lambda ch: plan[ch][0] - plan[ch][1])

            # phases 1+2 interleaved by token halves: kv chunks 0,1 need only
            # hT half-0, and the q0/attn block for tokens < 1024 (attention is
            # segment-block-diagonal, so those chunks only read k/v < 1024)
            # gives hT half-1 time to land on SP; norms/attn software-
            # pipelined so the PE queue never head-of-line blocks on the
            # ACT/DVE chains
            kvmm = {}
            q0mm = {}

            def kv_pair(j):
                kvmm[j] = (emit_proj_mm(j, 2), emit_proj_mm(j, 3))

            def kv_fin(j):
                emit_norm(j, 2, kvmm[j][0])
                emit_vtrans(j, kvmm[j][1])

            kv_pair(0)
            kv_pair(1)
            kv_fin(0)
            q0mm[0] = emit_proj_mm(0, 0)
            kv_fin(1)
            q0mm[1] = emit_proj_mm(1, 0)
            emit_norm(0, 0, q0mm[0])
            kv_pair(2)
            emit_norm(1, 0, q0mm[1])
            for ch in ch_by_j[0]:
                emit_attention(0, ch)
            kv_fin(2)
            kv_pair(3)
            for ch in ch_by_j[1]:
                emit_attention(0, ch)
            kv_fin(3)
            # Pool preloads for the gate phase, emitted after phase-1's Pool
            # work (hto + first gate columns + first wo tiles)
            nc.gpsimd.dma_start(hto_sb[:], hto_d[:])
            for g in range(8):
                w_ = wgsp.tile([P, DT, P], F16, tag="wgs", name=f"wg{g}")
                nc.gpsimd.dma_start(w_[:], wgp_d[:, g])
                wgs[g] = w_
            for ht in range(2):
                w_ = wop.tile([P, 2048], F16, tag="wop", bufs=3,
                              name=f"wo{ht}")
                nc.gpsimd.dma_start(w_[:], wo_d[:, ht, :])
                wo_slices[ht] = [w_[:, 512 * Dc:512 * Dc + 512]
                                 for Dc in range(NJ)]
            q0mm[2] = emit_proj_mm(2, 0)
            q0mm[3] = emit_proj_mm(3, 0)
            emit_norm(2, 0, q0mm[2])
            for ch in ch_by_j[2]:
                emit_attention(0, ch)
            emit_norm(3, 0, q0mm[3])
            for ch in ch_by_j[3]:
                emit_attention(0, ch)

            # phase 3: q1 per chunk; norm one chunk behind, attention two
            # chunks behind
            def phase_q(h):
                qmm = {}
                for j in range(NJ):
                    qmm[j] = emit_proj_mm(j, h)
                    if j >= 1:
                        emit_norm(j - 1, h, qmm[j - 1])
                    if j >= 2:
                        for ch in ch_by_j[j - 2]:
                            emit_attention(h, ch)
                emit_norm(NJ - 1, h, qmm[NJ - 1])
                for j in (NJ - 2, NJ - 1):
                    for ch in ch_by_j[j]:
                        emit_attention(h, ch)

            ATall = [None] * NT
            if use_collective:
                nc.gpsimd.collective_compute(
                    "AllToAll", OP.bypass,
                    replica_groups=[list(range(NCORES))],
                    ins=[a2a_in[0][:].opt()], outs=[a2a_out[0][:].opt()])
            else:
                nc.sync.dma_start(a2a_out[0][:], a2a_in[0][:])
            # ATall h0 loads on Pool right behind coll0: the sem-wait can't
            # block anything there (Pool has nothing live until coll1)
            for i in range(8):
                at_t = perm.tile([P, TSL], F16, tag="ATall", bufs=16,
                                 name=f"ATall{i}")
                nc.gpsimd.dma_start(at_t[:], a2a_out[0][P * i:P * i + P, :])
                ATall[i] = at_t

            phase_q(1)
            if use_collective:
                nc.gpsimd.collective_compute(
                    "AllToAll", OP.bypass,
                    replica_groups=[list(range(NCORES))],
                    ins=[a2a_in[1][:].opt()], outs=[a2a_out[1][:].opt()])
            else:
                nc.sync.dma_start(a2a_out[1][:], a2a_in[1][:])
            for i in range(8):
                at_t = perm.tile([P, TSL], F16, tag="ATall", bufs=16,
                                 name=f"ATall{8 + i}")
                nc.gpsimd.dma_start(at_t[:], a2a_out[1][P * i:P * i + P, :])
                ATall[8 + i] = at_t
            # SP is idle once the h1 staging is done: it takes the gate
            # weight tail and all remaining wo halves
            for g in range(8, NH):
                w_ = wgsp.tile([P, DT, P], F16, tag="wgs", name=f"wg{g}")
                nc.sync.dma_start(w_[:], wgp_d[:, g])
                wgs[g] = w_
            for ht in range(2, NT):
                sl = []
                for wh in range(2):
                    w_ = hw.tile([P, 1024], F16, tag="hw",
                                 name=f"wo_{ht}_{wh}")
                    nc.sync.dma_start(
                        w_[:], wo_d[:, ht, 1024 * wh:1024 * wh + 1024])
                    sl += [w_[:, 0:512], w_[:, 512:1024]]
                wo_slices[ht] = sl

            # ============ gate proj tail (g4..g15) ============
            # remaining wo tiles load on ACT interleaved with the sigmoid
            # drains
            for g in range(NH):
                emit_gate(g, exp_chain=(g < 3))

            # ================= o-proj =================
            ops_tags = ["mm", "mm", "mm", "aux", "acc", "acc", "acc", "acc"]
            ops_bufs = {"mm": 3, "aux": 1, "acc": 4}
            ops = []
            for m in range(2):
                for Dc in range(NJ):
                    tg = ops_tags[m * NJ + Dc]
                    ops.append(psp.tile([P, 512], F32, tag=tg,
                                        bufs=ops_bufs[tg], name=f"ops{m}_{Dc}"))
            for h in range(2):
                for i in range(8):
                    ht = 8 * h + i
                    # gate applied in-place on DVE (out aliases in1)
                    nc.vector.tensor_tensor(ATall[ht][:], sgT[:, ht, :],
                                            ATall[ht][:], OP.mult)
                for ht in range(8 * h, 8 * h + 8):
                    at_t = ATall[ht]
                    for m in range(2):
                        for Dc in range(NJ):
                            nc.tensor.matmul(
                                ops[m * NJ + Dc][:],
                                at_t[:, 128 * m:128 * m + 128],
                                wo_slices[ht][Dc],
                                start=(ht == 0), stop=(ht == NT - 1))

            # drain: 8 copies alternating DVE/ACT, 8 out DMAs on SP/Pool
            for m in range(2):
                for Dc in range(NJ):
                    idx = m * NJ + Dc
                    o_sb = osb.tile([P, 512], F16, tag="osb", bufs=8,
                                    name=f"o_{m}_{Dc}")
                    if idx % 2 == 0:
                        nc.vector.tensor_copy(o_sb[:], ops[idx][:])
                    else:
                        nc.scalar.activation(o_sb[:], ops[idx][:], AF.Copy)
                    eng = nc.sync if Dc % 2 == 0 else nc.gpsimd
                    eng.dma_start(
                        out_d[128 * m:128 * m + 128,
                              512 * Dc:512 * Dc + 512], o_sb[:])

    nc.compile()
    _dedupe_act_table_loads(nc)
    return nc


def _dedupe_act_table_loads(nc):
    """Minimize ACT table loads: start in natural_log_exp_and_others
    (Exp/Ln/Square/Copy) and switch tables (to sigmoid_and_friends and
    back) only at true transitions in the scheduled ACT order. The compiler
    inserts a load before every per-func table change, so a load always
    exists where a switch is needed; all others are dropped."""
    from concourse.hw_specs import get_activation_tables
    tabs = list(get_activation_tables(nc.m.arch).items())
    nl_exp = next(i for i, (nm, _) in enumerate(tabs)
                  if nm == "natural_log_exp_and_others")
    sig_id = next(i for i, (nm, _) in enumerate(tabs)
                  if nm == "sigmoid_and_friends")
    nl_funcs = tabs[nl_exp][1]
    sig_funcs = tabs[sig_id][1]

    kept_first = False
    current = nl_funcs
    n_loads = 0
    for bb in nc.main_func.blocks:
        keep = []
        pending = None
        for ins in bb.instructions:
            if isinstance(ins, mybir.InstLoadActFuncSet):
                assert ins.sync_info is None or (
                    not ins.sync_info.on_wait and not ins.sync_info.on_update)
                if not kept_first:
                    ins.act_func_set_id = nl_exp
                    keep.append(ins)
                    kept_first = True
                    n_loads += 1
                else:
                    pending = ins
                continue
            if isinstance(ins, mybir.InstActivation) and ins.func not in current:
                assert pending is not None, \
                    f"need table switch before {ins.func} but no load present"
                if ins.func in nl_funcs:
                    pending.act_func_set_id, current = nl_exp, nl_funcs
                elif ins.func in sig_funcs:
                    pending.act_func_set_id, current = sig_id, sig_funcs
                else:
                    raise AssertionError(f"func {ins.func} in no known table")
                keep.append(pending)
                pending = None
                n_loads += 1
            keep.append(ins)
        bb.instructions[:] = keep
    assert kept_first and n_loads <= 6, f"unexpected table load count {n_loads}"


def _host_prep(hidden_BTD, cos_BTK, sin_BTK, segment_ids_BT, position_ids_BT,
               wq, wk, wv, wo, q_norm_w, k_norm_w):
    hidden = np.ascontiguousarray(np.asarray(hidden_BTD, dtype=np.float32)[0])
    cos = np.asarray(cos_BTK, dtype=np.float32)[0]
    sin = np.asarray(sin_BTK, dtype=np.float32)[0]
    seg = np.asarray(segment_ids_BT)[0]
    pos = np.asarray(position_ids_BT)[0]
    wq = np.asarray(wq, dtype=np.float32)
    wk = np.asarray(wk, dtype=np.float32)
    wv = np.asarray(wv, dtype=np.float32)
    wo = np.asarray(wo, dtype=np.float32)
    q_norm_w = np.asarray(q_norm_w, dtype=np.float32)
    k_norm_w = np.asarray(k_norm_w, dtype=np.float32)

    assert np.array_equal(pos, np.arange(T, dtype=pos.dtype)), \
        "kernel assumes position_ids == arange"
    assert np.all(np.diff(seg) >= 0), "kernel assumes sorted segment ids"

    # [P, DT, T] fp16: hT[p, dt, t] = hidden[t, 128*dt + p]
    hT = np.ascontiguousarray(
        hidden.T.reshape(DT, P, T).transpose(1, 0, 2).astype(np.float16))
    sqrtS = np.float32(np.sqrt(SCALE))
    signv = np.where(np.arange(HD) < HD // 2, -1.0, 1.0).astype(np.float32)
    shuf = (np.arange(HD) + HD // 2) % HD

    cosw = (cos.T * sqrtS).astype(np.float32)
    sinw = (sin.T * signv[:, None] * sqrtS).astype(np.float32)
    sinswap = sinw[shuf]  # halves swapped: see rotate-half ops
    tblq = np.ascontiguousarray(np.stack([cosw, sinswap]).astype(np.float16))
    unit_w = bool(np.all(q_norm_w == 1.0) and np.all(k_norm_w == 1.0))
    wqk = np.ascontiguousarray(
        np.stack([q_norm_w, k_norm_w], axis=1).astype(np.float16))

    # o-proj / gate head order: even heads (h0 of each core) then odd
    permo = [2 * i + h for h in range(2) for i in range(NCORES)]
    wo_p = wo.reshape(NT, P, 2048)[permo].transpose(1, 0, 2)
    wo_p = np.ascontiguousarray(wo_p.astype(np.float16))
    # gate weights, [P, gi, dt, 128] with gi in permo order
    wgp = np.empty((P, NH, DT, P), dtype=np.float16)
    for gi, h in enumerate(permo):
        wg_h = wq[:, 256 * h + 128:256 * h + 256]          # [D, 128]
        wgp[:, gi] = wg_h.reshape(DT, P, P).transpose(1, 0, 2)
    wgp = np.ascontiguousarray(wgp)

    seg_end = np.searchsorted(seg, seg, side="right").astype(np.int64)
    iota = np.broadcast_to(
        np.arange(512, dtype=np.float16), (P, 512)).copy()
    plan = _tile_flags(seg_end)
    NCH = len(plan)
    segrel = np.zeros((P, NT, NCH), dtype=np.float16)
    caurel = np.zeros((P, NT, NCH), dtype=np.float16)
    rows = np.arange(P, dtype=np.float64)
    for ch, (t0, _, _) in enumerate(plan):
        for i in range(NT):
            segrel[:, i, ch] = (seg_end[P * i:P * i + P] - float(t0)).astype(
                np.float16)
            caurel[:, i, ch] = (P * i + rows - float(t0)).astype(np.float16)

    in_maps = []
    for c in range(NCORES):
        h0, h1 = 2 * c, 2 * c + 1
        g = c // 2
        wqc = np.concatenate([
            wq[:, h0 * 256: h0 * 256 + 128],
            wq[:, h1 * 256: h1 * 256 + 128],
        ], axis=1)
        wq_p = np.ascontiguousarray(
            wqc.reshape(DT, P, 256).transpose(1, 0, 2).astype(np.float16))
        wkv = np.concatenate([
            wk[:, g * 128:(g + 1) * 128], wv[:, g * 128:(g + 1) * 128]], axis=1)
        wkv_p = np.ascontiguousarray(
            wkv.reshape(DT, P, 256).transpose(1, 0, 2).astype(np.float16))
        hto = np.ascontiguousarray(hT[:, :, TSL * c:TSL * c + TSL])
        m = {
            "hT": hT, "wq": wq_p, "wkv": wkv_p, "wo": wo_p,
            "wgp": wgp, "hto": hto,
            "tblq": tblq, "iota": iota, "segrel": segrel, "caurel": caurel,
        }
        if not unit_w:
            m["wqk"] = wqk
        in_maps.append(m)
    return in_maps, seg_end, unit_w


def kernel(**inputs) -> np.ndarray:
    in_maps, seg_end, unit_w = _host_prep(**inputs)
    key = (_tile_flags(seg_end), unit_w)
    if key not in _program_cache:
        _program_cache[key] = _build_program(key)
    nc = _program_cache[key]
    res = run_bass_kernel_spmd(nc, in_maps, list(range(NCORES)))
    out = np.concatenate([res.results[c]["out"] for c in range(NCORES)], axis=0)
    return out[None].astype(np.float32)
